# revision 13
# baseline (speedup 1.0000x reference)
"""Trainium2 Bass kernel for nn_DecoderLayer (B=2,S=T=2048,E=1024,H=16,F=4096).

Sharding: token-parallel, no collectives. Core c handles batch b=c//4 and the
512-query slice starting at qpos=(c%4)*512 of that batch. K/V projections over
the batch's full 2048 tokens are replicated across the 4 cores of the batch
(K/V derive only from the kernel inputs x/enc, so no cross-core exchange is
needed). Everything runs in a transposed activation layout [feature, token] so
every matmul contracts over the partition axis with natural-layout DMAs; host
passes pre-transposed x/enc/weights. Causality is data-driven via a per-core
qidx vector so the same NEFF runs SPMD on all 8 cores.

Matmul dtype is float32r (~2x bf16 mantissa, measured ~1.5e-4 matmul rel-err)
except the attention-prob x V product which uses bf16. Scoresᵀ[k,q] layout
avoids every transpose: softmax denominators ride along the AV matmul as a
ones-column of [V|1] (even heads) / [1|V] (odd heads).
"""

import contextlib

import numpy as np

import concourse.bass as bass
import concourse.tile as tile
from concourse import mybir
from concourse.bass_utils import run_bass_kernel_spmd

B, S, T, E, H, F = 2, 2048, 2048, 1024, 16, 4096
HD = E // H
EPS = 1e-5
NCORES = 8
QS = 512  # queries per core
FB = E // 128  # feature blocks (8)
KB = S // 128  # key blocks (16)
HB = F // 128  # ffn hidden blocks (32)

f32 = mybir.dt.float32
f32r = mybir.dt.float32r
bf16 = mybir.dt.bfloat16
AF = mybir.ActivationFunctionType
ALU = mybir.AluOpType

W_NAMES = ["sa_qw", "sa_kw", "sa_vw", "sa_ow", "ca_qw", "ca_kw", "ca_vw", "ca_ow"]
B_NAMES = ["sa_qb", "sa_kb", "sa_vb", "sa_ob", "ca_qb", "ca_kb", "ca_vb", "ca_ob"]
LN_NAMES = ["ln1_g", "ln1_b", "ln2_g", "ln2_b", "ln3_g", "ln3_b"]


def _split_excess_waits(nc):
    """This container's walrus rejects >1 sync-wait per instruction (and any
    wait on Drain). Hoist excess waits onto single-wait NoOps just before the
    owning instruction on the same engine."""
    for fn in nc.m.functions:
        for bb in fn.blocks:
            new_list = []
            for ins in bb.instructions:
                si = getattr(ins, "sync_info", None)
                if si is not None and si.on_wait:
                    cap = 0 if ins.opcode in ("Drain",) else 1
                    if len(si.on_wait) > cap:
                        waits = list(si.on_wait)
                        keep = waits[-cap:] if cap else []
                        extra = waits[:-cap] if cap else waits
                        for w in extra:
                            new_list.append(mybir.InstNoOp(
                                name=f"I-splitw-{nc.next_id()}",
                                engine=ins.engine,
                                sync_info=mybir.SyncInfo(on_wait=[w], on_update=[]),
                                bass_nofuse=True,
                            ))
                        si.on_wait = keep
                new_list.append(ins)
            bb.instructions = new_list


def _bcast(src_ap, nparts):
    return bass.AP(tensor=src_ap.tensor, offset=src_ap.offset,
                   ap=[[0, nparts]] + list(src_ap.ap)[1:])


def _build_nc():
    nc = bass.Bass("TRN2", target_bir_lowering=False, debug=False,
                   num_devices=NCORES)

    xT = nc.dram_tensor("xT", [E, S], f32r, kind="ExternalInput")
    xqT = nc.dram_tensor("xqT", [E, QS], f32r, kind="ExternalInput")
    encT = nc.dram_tensor("encT", [E, T], f32r, kind="ExternalInput")
    qidx = nc.dram_tensor("qidx", [QS], f32r, kind="ExternalInput")
    wts = {n: nc.dram_tensor(n + "T", [E, E], f32r, kind="ExternalInput")
           for n in W_NAMES}
    w1T = nc.dram_tensor("w1T", [E, F], f32r, kind="ExternalInput")
    w2T = nc.dram_tensor("w2T", [F, E], f32r, kind="ExternalInput")
    bias = {n: nc.dram_tensor(n, [E], f32, kind="ExternalInput") for n in B_NAMES}
    b1d = nc.dram_tensor("b1", [F], f32, kind="ExternalInput")
    b2d = nc.dram_tensor("b2", [E], f32, kind="ExternalInput")
    lnd = {n: nc.dram_tensor(n, [E], f32, kind="ExternalInput") for n in LN_NAMES}
    outT = nc.dram_tensor("outT", [E, QS], f32, kind="ExternalOutput")

    def wcols(dram, c0, cn):
        # [E_in, out] dram -> [128, E_in/128, cn] partition-major view
        return dram.ap().rearrange("(eb p) d -> p eb d", p=128)[:, :, c0:c0 + cn]

    with tile.TileContext(nc) as tc:
        with contextlib.ExitStack() as ctx:
            constp = ctx.enter_context(tc.tile_pool(name="const", bufs=1))
            resp = ctx.enter_context(tc.tile_pool(name="res", bufs=1))
            wp = ctx.enter_context(tc.tile_pool(name="wp", bufs=2))
            xep = ctx.enter_context(tc.tile_pool(name="xep", bufs=1))
            tmpp = ctx.enter_context(tc.tile_pool(name="tmp", bufs=2))
            statp = ctx.enter_context(tc.tile_pool(name="stat", bufs=3))
            bcp = ctx.enter_context(tc.tile_pool(name="bc", bufs=1))
            obp = ctx.enter_context(tc.tile_pool(name="ob", bufs=2))
            pp = ctx.enter_context(tc.tile_pool(name="pp", bufs=4, space="PSUM"))
            avp = ctx.enter_context(tc.tile_pool(name="avp", bufs=2, space="PSUM"))
            lnpp = ctx.enter_context(tc.tile_pool(name="lnpp", bufs=1, space="PSUM"))
            drp = ctx.enter_context(tc.tile_pool(name="drp", bufs=2, space="DRAM"))

            def bcast_via_dram(dst_ap, src_row_ap, nparts, dt):
                sc = drp.tile([1, QS], dt, name="sc", tag="dr")
                nc.sync.dma_start(out=sc, in_=src_row_ap)
                nc.sync.dma_start(out=dst_ap, in_=bass.AP(
                    tensor=sc.tensor, offset=sc.offset, ap=[[0, nparts], [1, QS]]))

            # ---- packed constant vectors (one f32 tile, one f32r tile) ----
            NCV = HB + FB + 8 * FB + 6 * FB + 1
            cvec = constp.tile([128, NCV], f32)
            off = {}
            o = 0
            for nm, blks, dram in ([("b1", HB, b1d), ("b2", FB, b2d)]
                                   + [(n, FB, bias[n]) for n in B_NAMES]
                                   + [(n, FB, lnd[n]) for n in LN_NAMES]):
                off[nm] = o
                nc.sync.dma_start(out=cvec[:, o:o + blks],
                                  in_=dram.ap().rearrange("(b p) -> p b", p=128))
                o += blks
            off["eps"] = o
            nc.vector.memset(cvec[:, o:o + 1], EPS)

            def bv(nm, blk):
                return cvec[:, off[nm] + blk: off[nm] + blk + 1]

            crv = constp.tile([128, KB + 1], f32r)
            kidx_i = constp.tile([128, KB], mybir.dt.int32)
            nc.gpsimd.iota(kidx_i, pattern=[[128, KB]], base=0, channel_multiplier=1)
            nc.vector.tensor_copy(crv[:, 0:KB], kidx_i)
            nc.vector.memset(crv[:, KB:KB + 1].bitcast(f32), 1.0)
            kidx_f = crv[:, 0:KB]
            ones_col = crv[:, KB:KB + 1]

            qidx_b = constp.tile([128, QS], f32r)
            nc.sync.dma_start(out=qidx_b,
                              in_=bass.AP(tensor=qidx, offset=0, ap=[[0, 128], [1, QS]]))

            # ---- residents; xq, oT(sa), oT(ca), x2 share one slot ----
            x1 = resp.tile([128, FB, QS], f32r, tag="x1")

            def layernorm(src, gname, bname, dst=None, out_dram=None):
                ps_s = lnpp.tile([1, QS], f32, tag="lns")
                ps_q = lnpp.tile([1, QS], f32, tag="lnq")
                for f in range(FB):
                    sq = tmpp.tile([128, QS], f32r, tag="t")
                    nc.vector.tensor_mul(sq, src[:, f, :], src[:, f, :])
                    nc.tensor.matmul(ps_s, ones_col, src[:, f, :],
                                     start=(f == 0), stop=(f == FB - 1))
                    nc.tensor.matmul(ps_q, ones_col, sq,
                                     start=(f == 0), stop=(f == FB - 1))
                m_sb = statp.tile([1, QS], f32, name="m_sb", tag="st")
                q_sb = statp.tile([1, QS], f32, name="q_sb", tag="st")
                var = statp.tile([1, QS], f32, name="var", tag="st")
                nc.scalar.activation(m_sb, ps_s, AF.Copy, scale=1.0 / E)
                nc.scalar.activation(q_sb, ps_q, AF.Copy, scale=1.0 / E)
                nc.vector.tensor_mul(var, m_sb, m_sb)
                nc.vector.tensor_sub(var, q_sb, var)
                nc.scalar.activation(var, var, AF.Sqrt,
                                     bias=cvec[0:1, off["eps"]:off["eps"] + 1])
                nc.vector.reciprocal(var, var)
                m_b = bcp.tile([128, QS], f32, tag="mb")
                r_b = bcp.tile([128, QS], f32, tag="rb")
                bcast_via_dram(m_b, m_sb, 128, f32)
                bcast_via_dram(r_b, var, 128, f32)
                for f in range(FB):
                    t = tmpp.tile([128, QS], f32, tag="t")
                    nc.vector.tensor_sub(t, src[:, f, :], m_b)
                    nc.vector.tensor_mul(t, t, r_b)
                    if dst is not None:
                        nc.vector.tensor_scalar(dst[:, f, :], t, bv(gname, f),
                                                bv(bname, f), op0=ALU.mult, op1=ALU.add)
                    else:
                        ob = obp.tile([128, QS], f32, tag="outb")
                        nc.vector.tensor_scalar(ob, t, bv(gname, f),
                                                bv(bname, f), op0=ALU.mult, op1=ALU.add)
                        nc.sync.dma_start(out=out_dram[f * 128:(f + 1) * 128, :], in_=ob)

            with tc.tile_pool(name="kp", bufs=1) as kp, \
                 tc.tile_pool(name="vp", bufs=1) as vp, \
                 tc.tile_pool(name="app", bufs=2) as app, \
                 tc.tile_pool(name="rbp", bufs=2) as rbp:

                kT = kp.tile([128, FB, S], f32r)
                v = vp.tile([128, KB, H, HD + 1], bf16)

                # ones column of [V|1] for every head
                nc.vector.memset(v[:, :, :, HD:HD + 1], 1.0)

                def proj_K(w_dram, src_dram, bias_nm):
                    """kT[:, fb, t] = w.T @ srcT for all S tokens."""
                    for dq in range(4):
                        wh = wp.tile([128, FB, 256], f32r, tag="w")
                        nc.sync.dma_start(out=wh, in_=wcols(w_dram, dq * 256, 256))
                        for n in range(S // 512):
                            pss = [pp.tile([128, 512], f32, name=f"ps{_d}", tag="ps")
                                   for _d in range(2)]
                            for eh in range(2):
                                xe = xep.tile([128, FB // 2, 512], f32r, name="xe", tag="xe")
                                nc.sync.dma_start(
                                    out=xe,
                                    in_=src_dram.ap().rearrange("(eb p) t -> p eb t", p=128)[:, eh * 4:eh * 4 + 4, n * 512:(n + 1) * 512])
                                for e in range(FB // 2):
                                    for d in range(2):
                                        nc.tensor.matmul(pss[d], wh[:, eh * 4 + e, d * 128:(d + 1) * 128],
                                                         xe[:, e, :], start=(eh == 0 and e == 0), stop=(eh == 1 and e == FB // 2 - 1))
                            for d in range(2):
                                fb = dq * 2 + d
                                nc.scalar.activation(kT[:, fb, n * 512:(n + 1) * 512], pss[d],
                                                     AF.Identity, bias=bv(bias_nm, fb))

                def proj_Q(qT, w_dram, src_tile, bias_nm, src_dram=None):
                    for dq in range(4):
                        wh = wp.tile([128, FB, 256], f32r, tag="w")
                        nc.sync.dma_start(out=wh, in_=wcols(w_dram, dq * 256, 256))
                        pss = [pp.tile([128, QS], f32, name=f"ps{_d}", tag="ps")
                               for _d in range(2)]
                        if src_dram is not None:
                            for eh in range(2):
                                xqe = xep.tile([128, FB // 2, QS], f32r, name="xqe", tag="xe")
                                nc.sync.dma_start(out=xqe, in_=src_dram.ap().rearrange(
                                    "(eb p) q -> p eb q", p=128)[:, eh * 4:eh * 4 + 4, :])
                                for e in range(FB // 2):
                                    for d in range(2):
                                        nc.tensor.matmul(pss[d], wh[:, eh * 4 + e, d * 128:(d + 1) * 128],
                                                         xqe[:, e, :], start=(eh == 0 and e == 0), stop=(eh == 1 and e == FB // 2 - 1))
                        else:
                            for e in range(FB):
                                for d in range(2):
                                    nc.tensor.matmul(pss[d], wh[:, e, d * 128:(d + 1) * 128],
                                                     src_tile[:, e, :], start=(e == 0), stop=(e == FB - 1))
                        for d in range(2):
                            fb = dq * 2 + d
                            nc.scalar.activation(qT[:, fb, :], pss[d],
                                                 AF.Identity, bias=bv(bias_nm, fb))

                def proj_V(w_dram, src_dram):
                    """v[:, t, h, :] = (src @ w.T) in [token, head-dim] layout, bf16."""
                    for dq in range(4):
                        wh = wp.tile([128, FB, 256], f32r, tag="w")
                        nc.sync.dma_start(out=wh, in_=wcols(w_dram, dq * 256, 256))
                        for t in range(KB):
                            xv = xep.tile([128, FB, 128], f32r, tag="xe")
                            nc.sync.dma_start(
                                out=xv,
                                in_=src_dram.ap().rearrange("(eb p) t -> p eb t", p=128)[:, :, t * 128:(t + 1) * 128])
                            ps = pp.tile([128, 256], f32, tag="ps")
                            for e in range(FB):
                                nc.tensor.matmul(ps, xv[:, e, :], wh[:, e, :],
                                                 start=(e == 0), stop=(e == FB - 1))
                            pv = ps.rearrange("p (h d) -> p h d", d=HD)
                            nc.vector.tensor_copy(v[:, t, 4 * dq:4 * dq + 4, 0:HD], pv)

                def attention(qT, oT, masked, vbias_nm):
                    for h in range(H):
                        base = (h % 2) * HD
                        fb = h // 2
                        av = avp.tile([128, QS], f32, tag="av")
                        for kb in range(KB):
                            sps = pp.tile([128, QS], f32, tag="ps")
                            nc.tensor.matmul(sps, kT[base:base + HD, fb, kb * 128:(kb + 1) * 128],
                                             qT[base:base + HD, fb, :], start=True, stop=True)
                            at = app.tile([128, QS], bf16, tag="a")
                            nc.scalar.activation(at, sps, AF.Exp,
                                                 scale=float(1.0 / np.sqrt(HD)))
                            if masked:
                                nc.vector.scalar_tensor_tensor(
                                    at, qidx_b, kidx_f[:, kb:kb + 1], at,
                                    op0=ALU.is_ge, op1=ALU.mult)
                            nc.tensor.matmul(av[0:HD + 1, :], v[:, kb, h, :], at,
                                             start=(kb == 0), stop=(kb == KB - 1))
                        rc = rbp.tile([128, QS], f32, tag="rc")
                        nc.vector.reciprocal(rc[HD:HD + 1, :], av[HD:HD + 1, :])
                        rb = rbp.tile([128, QS], f32, tag="rb")
                        bcast_via_dram(rb[base:base + HD, :], rc[HD:HD + 1, :], HD, f32)
                        nc.vector.tensor_mul(oT[base:base + HD, fb, :], av[0:HD, :],
                                             rb[base:base + HD, :])
                        nc.vector.tensor_scalar_add(
                            oT[base:base + HD, fb, :], oT[base:base + HD, fb, :],
                            cvec[base:base + HD, off[vbias_nm] + fb: off[vbias_nm] + fb + 1])

                def proj_O(oT, s_res, w_dram, bias_nm, res_tile, res_dram=None):
                    """s_res = w.T @ oT + bias + residual."""
                    for dq in range(4):
                        wh = wp.tile([128, FB, 256], f32r, tag="w")
                        nc.sync.dma_start(out=wh, in_=wcols(w_dram, dq * 256, 256))
                        pss = [pp.tile([128, QS], f32, name=f"ps{_d}", tag="ps")
                               for _d in range(2)]
                        for f in range(FB):
                            for d in range(2):
                                nc.tensor.matmul(pss[d], wh[:, f, d * 128:(d + 1) * 128],
                                                 oT[:, f, :], start=(f == 0), stop=(f == FB - 1))
                        for d in range(2):
                            fb = dq * 2 + d
                            t = tmpp.tile([128, QS], f32r, tag="t")
                            nc.scalar.activation(t, pss[d], AF.Identity,
                                                 bias=bv(bias_nm, fb))
                            if res_dram is not None:
                                rt = tmpp.tile([128, QS], f32r, name="rt", tag="t")
                                nc.sync.dma_start(out=rt, in_=res_dram.ap().rearrange(
                                    "(eb p) q -> p eb q", p=128)[:, fb, :])
                                nc.vector.tensor_add(s_res[:, fb, :], t, rt)
                            else:
                                nc.vector.tensor_add(s_res[:, fb, :], t, res_tile[:, fb, :])

                # ---- self attention ----
                proj_K(wts["sa_kw"], xT, "sa_kb")
                proj_V(wts["sa_vw"], xT)
                qT_sa = resp.tile([128, FB, QS], f32r, name="qT_sa", tag="qs")
                proj_Q(qT_sa, wts["sa_qw"], None, "sa_qb", src_dram=xqT)
                oT_sa = resp.tile([128, FB, QS], f32r, name="oT_sa", tag="xo")
                attention(qT_sa, oT_sa, masked=True, vbias_nm="sa_vb")
                s_sa = resp.tile([128, FB, QS], f32r, name="s_sa", tag="qs")
                proj_O(oT_sa, s_sa, wts["sa_ow"], "sa_ob", None, res_dram=xqT)
                layernorm(s_sa, "ln1_g", "ln1_b", dst=x1)

                # ---- cross attention ----
                proj_K(wts["ca_kw"], encT, "ca_kb")
                proj_V(wts["ca_vw"], encT)
                qT_ca = resp.tile([128, FB, QS], f32r, name="qT_ca", tag="qs")
                proj_Q(qT_ca, wts["ca_qw"], x1, "ca_qb")
                oT_ca = resp.tile([128, FB, QS], f32r, name="oT_ca", tag="xo")
                attention(qT_ca, oT_ca, masked=False, vbias_nm="ca_vb")
                s_ca = resp.tile([128, FB, QS], f32r, name="s_ca", tag="qs")
                proj_O(oT_ca, s_ca, wts["ca_ow"], "ca_ob", x1)
                x2 = resp.tile([128, FB, QS], f32r, name="x2", tag="xo")
                layernorm(s_ca, "ln2_g", "ln2_b", dst=x2)

                # ---- FFN (inside attn pools' scope would waste SBUF; but x2
                # lives in resp, and hT replaces kT's budget once kp closes) ----

            with tc.tile_pool(name="hp", bufs=1) as hp, \
                 tc.tile_pool(name="fw1", bufs=3) as fw1p, \
                 tc.tile_pool(name="fw2", bufs=2) as fw2p:
                hT = hp.tile([128, HB, QS], f32r)
                s_ffn = resp.tile([128, FB, QS], f32r, name="s_ffn", tag="qs")
                for hb in range(HB):
                    fw = fw1p.tile([128, FB, 128], f32r, tag="f1")
                    nc.sync.dma_start(out=fw, in_=wcols(w1T, hb * 128, 128))
                    ps = pp.tile([128, QS], f32, tag="ps")
                    for e in range(FB):
                        nc.tensor.matmul(ps, fw[:, e, :], x2[:, e, :],
                                         start=(e == 0), stop=(e == FB - 1))
                    nc.scalar.activation(hT[:, hb, :], ps, AF.Relu, bias=bv("b1", hb))
                for ob in range(FB):
                    fw2 = fw2p.tile([128, HB, 128], f32r, tag="f2")
                    nc.sync.dma_start(out=fw2, in_=w2T.ap().rearrange(
                        "(hb p) d -> p hb d", p=128)[:, :, ob * 128:(ob + 1) * 128])
                    ps = pp.tile([128, QS], f32, tag="ps")
                    for hb in range(HB):
                        nc.tensor.matmul(ps, fw2[:, hb, :], hT[:, hb, :],
                                         start=(hb == 0), stop=(hb == HB - 1))
                    t = tmpp.tile([128, QS], f32r, tag="t")
                    nc.scalar.activation(t, ps, AF.Identity, bias=bv("b2", ob))
                    nc.vector.tensor_add(s_ffn[:, ob, :], t, x2[:, ob, :])

                layernorm(s_ffn, "ln3_g", "ln3_b", out_dram=outT.ap())

    _split_excess_waits(nc)
    return nc


_NC_CACHE = []


def _get_nc():
    if not _NC_CACHE:
        _NC_CACHE.append(_build_nc())
    return _NC_CACHE[0]


def kernel(**inputs):
    inputs = {k: np.asarray(v) for k, v in inputs.items()}
    x = inputs["x"].astype(np.float32)
    enc = inputs["enc"].astype(np.float32)

    xT_b = [np.ascontiguousarray(x[b].T) for b in range(B)]
    encT_b = [np.ascontiguousarray(enc[b].T) for b in range(B)]
    shared = {}
    for n in W_NAMES:
        shared[n + "T"] = np.ascontiguousarray(inputs[n].astype(np.float32).T)
    shared["w1T"] = np.ascontiguousarray(inputs["w1"].astype(np.float32).T)
    shared["w2T"] = np.ascontiguousarray(inputs["w2"].astype(np.float32).T)
    for n in B_NAMES:
        shared[n] = inputs[n].astype(np.float32)
    shared["b1"] = inputs["b1"].astype(np.float32)
    shared["b2"] = inputs["b2"].astype(np.float32)
    for n in LN_NAMES:
        shared[n] = inputs[n].astype(np.float32)

    in_maps = []
    for c in range(NCORES):
        b, qpos = c // 4, (c % 4) * QS
        im = dict(shared)
        im["xT"] = xT_b[b]
        im["encT"] = encT_b[b]
        im["xqT"] = np.ascontiguousarray(xT_b[b][:, qpos:qpos + QS])
        im["qidx"] = (qpos + np.arange(QS)).astype(np.float32)
        in_maps.append(im)

    nc = _get_nc()
    res = run_bass_kernel_spmd(nc, in_maps, core_ids=list(range(NCORES)))
    out = np.empty((B, S, E), np.float32)
    for c in range(NCORES):
        b, qpos = c // 4, (c % 4) * QS
        out[b, qpos:qpos + QS, :] = res.results[c]["outT"].T
    return out
